# revision 9
# baseline (speedup 1.0000x reference)
"""Trainium2 Bass kernel for nn_BaseAtt (attention pooling) — v3.

Like v2.3 but the d-major nf operand for the alpha matmuls is built
ON-CHIP by per-batch PE transposes of the k-major x0/x1 tiles instead of
loading a second 26 MB copy of nf from HBM.  ~55 MB HBM traffic per core.

Per block (32 batches), per group of 4 batches:
  - 8 PE transposes (x0 nf [128k,128d] -> [128d,128k], x1 nf [72,128] ->
    [128,72]) into one PSUM bank tile [128, 4, 200] f16
  - one PSUM->SBUF copy (alternating DVE/ACT)
  - 4 alpha matmuls (zero-masked stationary z trick) vs the copied tiles
Rest identical to v2.3.
"""

import numpy as np

B, K, D, FD = 4096, 200, 128, 1024
NCORES = 8
BC = B // NCORES          # 512 batches per core
BLK = 32                  # block size (batches per block)
HB = BLK // 2             # half block
NBLK = BC // BLK
K0, K1 = 128, K - 128     # k-chunk sizes (128 + 72)
GRP = 4                   # batches per transpose group
NEG = -1.0e9              # mask fill for logits (exp -> 0)


def gen_kernel():
    import concourse.bacc as bacc
    import concourse.tile as tile
    from concourse import mybir

    f32 = mybir.dt.float32
    f16 = mybir.dt.float16
    AX = mybir.AxisListType
    AF = mybir.ActivationFunctionType

    nc = bacc.Bacc()

    tft = nc.declare_dram_parameter("tft", [128, 8, BC], f16, isOutput=False)
    wt = nc.declare_dram_parameter("wt", [128, 8, D], f16, isOutput=False)
    xh = nc.declare_dram_parameter("xh", [K, BC, 2 * D], f16, isOutput=False)
    lmask = nc.declare_dram_parameter("lmask", [BC, K], f32, isOutput=False)
    ident = nc.declare_dram_parameter("ident", [128, 128], f16, isOutput=False)
    m32h = nc.declare_dram_parameter("m32h", [128, BLK, BLK], f16, isOutput=False)

    ofull = nc.declare_dram_parameter("ofull", [BC, 2 * D], f32, isOutput=True)

    with tile.TileContext(nc) as tc:
        with (
            tc.tile_pool(name="const", bufs=1) as const,
            tc.tile_pool(name="xin", bufs=4) as xin,
            tc.tile_pool(name="ntr", bufs=3) as ntr,
            tc.tile_pool(name="sm", bufs=3) as sm,
            tc.tile_pool(name="lmp", bufs=4) as lmp,
            tc.tile_pool(name="zp", bufs=2) as zp,
            tc.tile_pool(name="outp", bufs=4) as outp,
            tc.tile_pool(name="psa", bufs=2, space="PSUM") as psa,
            tc.tile_pool(name="pso", bufs=2, space="PSUM") as pso,
            tc.tile_pool(name="psx", bufs=1, space="PSUM") as psx,
            tc.tile_pool(name="pstr", bufs=2, space="PSUM") as pstr,
        ):
            # ---- setup: constants (projection inputs first in the queues) ----
            wt_t = const.tile([128, 8, D], f16)
            nc.scalar.dma_start(out=wt_t, in_=wt[:, :, :])
            tft_t = const.tile([128, 8, BC], f16)
            nc.scalar.dma_start(out=tft_t, in_=tft[:, :, :])
            id_t = const.tile([128, 128], f16)
            nc.gpsimd.dma_start(out=id_t, in_=ident[:, :])
            m32h_t = const.tile([128, BLK, BLK], f16)
            nc.gpsimd.dma_start(out=m32h_t, in_=m32h[:, :, :])

            # ---- target.T = W @ tf.T : [128 d, BC b] ----
            with tc.tile_pool(name="pst", bufs=1, space="PSUM") as pst:
                ps_t = pst.tile([128, BC], f32)
                for fb in range(8):
                    nc.tensor.matmul(
                        ps_t, wt_t[:, fb, :], tft_t[:, fb, :],
                        start=(fb == 0), stop=(fb == 7),
                    )
                targetT = const.tile([128, BC], f16)
                nc.vector.tensor_copy(out=targetT, in_=ps_t)

            def load_xw(bb):
                """x loads for block bb (two half tiles per k-chunk):
                x0 halves alternate between the two HWDGE queues (sync and
                scalar) to keep both rings feeding the 16 SDMA engines;
                the 72-partition x1 rides gpsimd-SWDGE."""
                b0 = bb * BLK
                xs = []
                for h in range(2):
                    x0 = xin.tile([K0, HB, 256], f16, tag=f"x0{h}")
                    eng = nc.sync if (2 * bb + h) % 2 == 0 else nc.scalar
                    eng.dma_start(
                        out=x0, in_=xh[0:K0, b0 + h * HB : b0 + (h + 1) * HB, :]
                    )
                    x1 = xin.tile([K1, HB, 256], f16, tag=f"x1{h}")
                    nc.gpsimd.dma_start(
                        out=x1, in_=xh[K0:K, b0 + h * HB : b0 + (h + 1) * HB, :]
                    )
                    xs.append((x0, x1))
                return xs

            def build_z(bb):
                b0 = bb * BLK
                z_t = zp.tile([128, BLK, BLK], f16, tag="z")
                nc.vector.tensor_mul(
                    out=z_t,
                    in0=targetT[:, b0 : b0 + BLK].unsqueeze(2).broadcast_to(
                        [128, BLK, BLK]
                    ),
                    in1=m32h_t,
                )
                return z_t

            def alpha_phase(bb, z_t, sm_prev, xs):
                """Per group of 4 batches: 8 PE transposes -> PSUM, one copy
                to SBUF, then 4 alpha matmuls vs the transposed tiles.
                Group g's alpha MMs are issued after group g+1's transposes
                so the PE never waits on the copy engines."""
                b0 = bb * BLK
                lm_t = lmp.tile([BLK, K], f32, tag="lm")
                nc.gpsimd.dma_start(out=lm_t, in_=lmask[b0 : b0 + BLK, :])
                ps_a = psa.tile([BLK, 256], f32, tag="psa")

                def transpose_group(g):
                    psT = pstr.tile([128, GRP, K], f16, tag="tr")
                    for q in range(GRP):
                        i = g * GRP + q
                        x0, x1 = xs[i // HB]
                        j = i % HB
                        nc.tensor.transpose(
                            psT[:, q, 0:K0], x0[:, j, 0:D], id_t
                        )
                        nc.tensor.transpose(
                            psT[:, q, K0:K], x1[:, j, 0:D], id_t[:K1, :K1]
                        )
                    nfT = ntr.tile([128, GRP, K], f16, tag="nfT")
                    if g % 2 == 0:
                        nc.vector.tensor_copy(out=nfT, in_=psT)
                    else:
                        nc.scalar.copy(out=nfT, in_=psT)
                    return nfT

                def alpha_group(g, nfT):
                    for q in range(GRP):
                        i = g * GRP + q
                        nc.tensor.matmul(
                            ps_a[:, 0:K], z_t[:, i, :], nfT[:, q, :],
                            start=(i == 0), stop=(i == BLK - 1),
                        )

                w_ready = None
                ngrp = BLK // GRP
                nfT_prev = transpose_group(0)
                for g in range(1, ngrp):
                    nfT_cur = transpose_group(g)
                    if g == 2 and sm_prev is not None:
                        w_ready = prep_weighted(sm_prev)
                    alpha_group(g - 1, nfT_prev)
                    nfT_prev = nfT_cur
                alpha_group(ngrp - 1, nfT_prev)
                if sm_prev is not None and w_ready is None:
                    w_ready = prep_weighted(sm_prev)
                return b0, ps_a, xs, lm_t, w_ready

            def softmax_phase(state):
                b0, ps_a, xs, lm_t = state
                aM = sm.tile([BLK, K], f32, tag="am")
                nc.vector.tensor_add(out=aM, in0=ps_a[:, 0:K], in1=lm_t)
                mx = sm.tile([BLK, 1], f32, tag="mx")
                nc.vector.reduce_max(out=mx, in_=aM, axis=AX.X)
                negmx = sm.tile([BLK, 1], f32, tag="negmx")
                nc.vector.tensor_scalar_mul(out=negmx, in0=mx, scalar1=-1.0)
                aE = sm.tile([BLK, K], f32, tag="ae")
                s_t = sm.tile([BLK, 1], f32, tag="s")
                nc.scalar.activation(
                    out=aE, in_=aM, func=AF.Exp, bias=negmx, scale=1.0,
                    accum_out=s_t,
                )
                rs = sm.tile([BLK, 1], f32, tag="rs")
                nc.vector.reciprocal(out=rs, in_=s_t)
                aN = sm.tile([BLK, K], f16, tag="an")
                nc.vector.tensor_scalar_mul(out=aN, in0=aE, scalar1=rs)
                return b0, aN, xs

            def prep_weighted(smstate):
                b0, aN, xs = smstate
                ps_aT = psx.tile([128, 2, BLK], f16, tag="pat")
                nc.tensor.transpose(ps_aT[:, 0, :], aN[:, 0:K0], id_t[:BLK, :BLK])
                nc.tensor.transpose(
                    ps_aT[:K1, 1, :], aN[:, K0:K], id_t[:BLK, :BLK]
                )
                aTb = zp.tile([128, 2, BLK], f16, tag="atb")
                nc.scalar.copy(out=aTb, in_=ps_aT)

                za0 = zp.tile([128, BLK, BLK], f16, tag="za0")
                nc.vector.tensor_mul(
                    out=za0,
                    in0=aTb[:, 0, :].unsqueeze(2).broadcast_to([128, BLK, BLK]),
                    in1=m32h_t,
                )
                za1 = zp.tile([K1, BLK, BLK], f16, tag="za1")
                nc.vector.tensor_mul(
                    out=za1,
                    in0=aTb[:K1, 1, :].unsqueeze(2).broadcast_to(
                        [K1, BLK, BLK]
                    ),
                    in1=m32h_t[:K1],
                )
                return b0, xs, za0, za1

            def weighted_phase(wstate):
                b0, xs, za0, za1 = wstate
                ps_o = pso.tile([BLK, 256], f32, tag="pso")
                for i in range(BLK):
                    x0, x1 = xs[i // HB]
                    j = i % HB
                    nc.tensor.matmul(
                        ps_o, za0[:, i, :], x0[:, j, :],
                        start=(i == 0), stop=False,
                    )
                    nc.tensor.matmul(
                        ps_o, za1[:, i, :], x1[:, j, :],
                        start=False, stop=(i == BLK - 1),
                    )
                out_s = outp.tile([BLK, 256], f32, tag="outs")
                nc.vector.tensor_copy(out=out_s[:, 0:128], in_=ps_o[:, 0:128])
                nc.scalar.copy(out=out_s[:, 128:256], in_=ps_o[:, 128:256])
                nc.gpsimd.dma_start(
                    out=ofull[b0 : b0 + BLK, :], in_=out_s
                )

            # ---- software-pipelined main loop ----
            xpre = {0: load_xw(0), 1: load_xw(1)}
            sm_prev = None
            z_next = build_z(0)
            for bb in range(NBLK):
                z_cur = z_next
                b0, ps_a, xs, lm_t, w_ready = alpha_phase(
                    bb, z_cur, sm_prev, xpre.pop(bb)
                )
                if bb + 1 < NBLK:
                    z_next = build_z(bb + 1)
                if w_ready is not None:
                    weighted_phase(w_ready)
                sm_prev = softmax_phase((b0, ps_a, xs, lm_t))
                if bb + 2 < NBLK:
                    xpre[bb + 2] = load_xw(bb + 2)
            weighted_phase(prep_weighted(sm_prev))

    nc.finalize()
    return nc


_NC_CACHE = None


def _get_nc():
    global _NC_CACHE
    if _NC_CACHE is None:
        _NC_CACHE = gen_kernel()
    return _NC_CACHE


def build_in_maps(target_feats, neighbor_feats, neighbor_label, hist_mask, W):
    target_feats = np.ascontiguousarray(target_feats, dtype=np.float32)
    neighbor_feats = np.ascontiguousarray(neighbor_feats, dtype=np.float32)
    neighbor_label = np.ascontiguousarray(neighbor_label, dtype=np.float32)
    W = np.ascontiguousarray(W, dtype=np.float32)

    # [128, 8, D]: contiguous per-partition runs (one descriptor each)
    wt_full = np.ascontiguousarray(
        W.T.reshape(8, 128, D).transpose(1, 0, 2)
    ).astype(np.float16)
    lmask_full = np.where(np.asarray(hist_mask) > 0, 0.0, NEG).astype(np.float32)
    ident = np.eye(128, dtype=np.float16)
    m32 = np.zeros((128, BLK, BLK), dtype=np.float16)
    for i in range(BLK):
        m32[:, i, i] = 1.0

    in_maps = []
    for c in range(NCORES):
        s = slice(c * BC, (c + 1) * BC)
        xh = np.empty((K, BC, 2 * D), dtype=np.float16)
        xh[:, :, 0:D] = neighbor_feats[s].transpose(1, 0, 2)
        xh[:, :, D : 2 * D] = neighbor_label[s].transpose(1, 0, 2)
        in_maps.append({
            "tft": np.ascontiguousarray(
                target_feats[s].T.reshape(8, 128, BC).transpose(1, 0, 2)
            ).astype(np.float16),
            "wt": wt_full,
            "xh": xh,
            "lmask": lmask_full[s],
            "ident": ident,
            "m32h": m32,
        })
    return in_maps


def kernel(target_feats, neighbor_feats, neighbor_label, hist_mask, W):
    from concourse.bass_utils import run_bass_kernel_spmd

    in_maps = build_in_maps(
        target_feats, neighbor_feats, neighbor_label, hist_mask, W
    )
    nc = _get_nc()
    res = run_bass_kernel_spmd(nc, in_maps, list(range(NCORES))).results

    ofull = np.concatenate([res[c]["ofull"] for c in range(NCORES)], axis=0)
    return np.ascontiguousarray(ofull[:, 0:D]), np.ascontiguousarray(
        ofull[:, D : 2 * D]
    )


# revision 10
# speedup vs baseline: 1.2196x; 1.2196x over previous
"""Trainium2 Bass kernel for nn_BaseAtt (attention pooling) — v3.

Like v2.3 but the d-major nf operand for the alpha matmuls is built
ON-CHIP by per-batch PE transposes of the k-major x0/x1 tiles instead of
loading a second 26 MB copy of nf from HBM.  ~55 MB HBM traffic per core.

Per block (32 batches), per group of 4 batches:
  - 8 PE transposes (x0 nf [128k,128d] -> [128d,128k], x1 nf [72,128] ->
    [128,72]) into one PSUM bank tile [128, 4, 200] f16
  - one PSUM->SBUF copy (alternating DVE/ACT)
  - 4 alpha matmuls (zero-masked stationary z trick) vs the copied tiles
Rest identical to v2.3.
"""

import numpy as np

B, K, D, FD = 4096, 200, 128, 1024
NCORES = 8
BC = B // NCORES          # 512 batches per core
BLK = 32                  # block size (batches per block)
HB = BLK // 2             # half block
NBLK = BC // BLK
K0, K1 = 128, K - 128     # k-chunk sizes (128 + 72)
GRP = 4                   # batches per transpose group
NEG = -1.0e9              # mask fill for logits (exp -> 0)


def gen_kernel():
    import concourse.bacc as bacc
    import concourse.tile as tile
    from concourse import mybir

    f32 = mybir.dt.float32
    f16 = mybir.dt.float16
    AX = mybir.AxisListType
    AF = mybir.ActivationFunctionType

    nc = bacc.Bacc()

    tft = nc.declare_dram_parameter("tft", [128, 8, BC], f16, isOutput=False)
    wt = nc.declare_dram_parameter("wt", [128, 8, D], f16, isOutput=False)
    xh = nc.declare_dram_parameter("xh", [K, BC, 2 * D], f16, isOutput=False)
    lmask = nc.declare_dram_parameter("lmask", [BC, K], f32, isOutput=False)
    ident = nc.declare_dram_parameter("ident", [128, 128], f16, isOutput=False)
    m32h = nc.declare_dram_parameter("m32h", [128, BLK, BLK], f16, isOutput=False)

    ofull = nc.declare_dram_parameter("ofull", [BC, 2 * D], f32, isOutput=True)

    with tile.TileContext(nc) as tc:
        with (
            tc.tile_pool(name="const", bufs=1) as const,
            tc.tile_pool(name="xin", bufs=4) as xin,
            tc.tile_pool(name="ntr", bufs=3) as ntr,
            tc.tile_pool(name="sm", bufs=3) as sm,
            tc.tile_pool(name="lmp", bufs=4) as lmp,
            tc.tile_pool(name="zp", bufs=2) as zp,
            tc.tile_pool(name="outp", bufs=4) as outp,
            tc.tile_pool(name="psa", bufs=2, space="PSUM") as psa,
            tc.tile_pool(name="pso", bufs=2, space="PSUM") as pso,
            tc.tile_pool(name="psx", bufs=1, space="PSUM") as psx,
            tc.tile_pool(name="pstr", bufs=2, space="PSUM") as pstr,
        ):
            # ---- setup: constants (projection inputs first in the queues) ----
            wt_t = const.tile([128, 8, D], f16)
            nc.scalar.dma_start(out=wt_t, in_=wt[:, :, :])
            tft_t = const.tile([128, 8, BC], f16)
            nc.scalar.dma_start(out=tft_t, in_=tft[:, :, :])
            id_t = const.tile([128, 128], f16)
            nc.gpsimd.dma_start(out=id_t, in_=ident[:, :])
            m32h_t = const.tile([128, BLK, BLK], f16)
            nc.gpsimd.dma_start(out=m32h_t, in_=m32h[:, :, :])

            # ---- target.T = W @ tf.T : [128 d, BC b] ----
            with tc.tile_pool(name="pst", bufs=1, space="PSUM") as pst:
                ps_t = pst.tile([128, BC], f32)
                for fb in range(8):
                    nc.tensor.matmul(
                        ps_t, wt_t[:, fb, :], tft_t[:, fb, :],
                        start=(fb == 0), stop=(fb == 7),
                    )
                targetT = const.tile([128, BC], f16)
                nc.vector.tensor_copy(out=targetT, in_=ps_t)

            def load_xw(bb):
                """x loads for block bb (two half tiles per k-chunk):
                x0 halves alternate between the two HWDGE queues (sync and
                scalar) to keep both rings feeding the 16 SDMA engines;
                the 72-partition x1 rides gpsimd-SWDGE."""
                b0 = bb * BLK
                xs = []
                for h in range(2):
                    x0 = xin.tile([K0, HB, 256], f16, tag=f"x0{h}")
                    nc.sync.dma_start(
                        out=x0, in_=xh[0:K0, b0 + h * HB : b0 + (h + 1) * HB, :]
                    )
                    x1 = xin.tile([K1, HB, 256], f16, tag=f"x1{h}")
                    nc.gpsimd.dma_start(
                        out=x1, in_=xh[K0:K, b0 + h * HB : b0 + (h + 1) * HB, :]
                    )
                    xs.append((x0, x1))
                return xs

            def build_z(bb):
                b0 = bb * BLK
                z_t = zp.tile([128, BLK, BLK], f16, tag="z")
                nc.vector.tensor_mul(
                    out=z_t,
                    in0=targetT[:, b0 : b0 + BLK].unsqueeze(2).broadcast_to(
                        [128, BLK, BLK]
                    ),
                    in1=m32h_t,
                )
                return z_t

            def alpha_phase(bb, z_t, sm_prev, xs):
                """Per group of 4 batches: 8 PE transposes -> PSUM, one copy
                to SBUF, then 4 alpha matmuls vs the transposed tiles.
                Group g's alpha MMs are issued after group g+1's transposes
                so the PE never waits on the copy engines."""
                b0 = bb * BLK
                lm_t = lmp.tile([BLK, K], f32, tag="lm")
                nc.gpsimd.dma_start(out=lm_t, in_=lmask[b0 : b0 + BLK, :])
                ps_a = psa.tile([BLK, 256], f32, tag="psa")

                def transpose_group(g):
                    psT = pstr.tile([128, GRP, K], f16, tag="tr")
                    for q in range(GRP):
                        i = g * GRP + q
                        x0, x1 = xs[i // HB]
                        j = i % HB
                        nc.tensor.transpose(
                            psT[:, q, 0:K0], x0[:, j, 0:D], id_t
                        )
                        nc.tensor.transpose(
                            psT[:, q, K0:K], x1[:, j, 0:D], id_t[:K1, :K1]
                        )
                    nfT = ntr.tile([128, GRP, K], f16, tag="nfT")
                    if g % 2 == 0:
                        nc.vector.tensor_copy(out=nfT, in_=psT)
                    else:
                        nc.scalar.copy(out=nfT, in_=psT)
                    return nfT

                def alpha_group(g, nfT):
                    for q in range(GRP):
                        i = g * GRP + q
                        nc.tensor.matmul(
                            ps_a[:, 0:K], z_t[:, i, :], nfT[:, q, :],
                            start=(i == 0), stop=(i == BLK - 1),
                        )

                w_ready = None
                ngrp = BLK // GRP
                nfT_prev = transpose_group(0)
                for g in range(1, ngrp):
                    nfT_cur = transpose_group(g)
                    if g == 2 and sm_prev is not None:
                        w_ready = prep_weighted(sm_prev)
                    alpha_group(g - 1, nfT_prev)
                    nfT_prev = nfT_cur
                alpha_group(ngrp - 1, nfT_prev)
                if sm_prev is not None and w_ready is None:
                    w_ready = prep_weighted(sm_prev)
                return b0, ps_a, xs, lm_t, w_ready

            def softmax_phase(state):
                b0, ps_a, xs, lm_t = state
                aM = sm.tile([BLK, K], f32, tag="am")
                nc.vector.tensor_add(out=aM, in0=ps_a[:, 0:K], in1=lm_t)
                mx = sm.tile([BLK, 1], f32, tag="mx")
                nc.vector.reduce_max(out=mx, in_=aM, axis=AX.X)
                negmx = sm.tile([BLK, 1], f32, tag="negmx")
                nc.vector.tensor_scalar_mul(out=negmx, in0=mx, scalar1=-1.0)
                aE = sm.tile([BLK, K], f32, tag="ae")
                s_t = sm.tile([BLK, 1], f32, tag="s")
                nc.scalar.activation(
                    out=aE, in_=aM, func=AF.Exp, bias=negmx, scale=1.0,
                    accum_out=s_t,
                )
                rs = sm.tile([BLK, 1], f32, tag="rs")
                nc.vector.reciprocal(out=rs, in_=s_t)
                aN = sm.tile([BLK, K], f16, tag="an")
                nc.vector.tensor_scalar_mul(out=aN, in0=aE, scalar1=rs)
                return b0, aN, xs

            def prep_weighted(smstate):
                b0, aN, xs = smstate
                ps_aT = psx.tile([128, 2, BLK], f16, tag="pat")
                nc.tensor.transpose(ps_aT[:, 0, :], aN[:, 0:K0], id_t[:BLK, :BLK])
                nc.tensor.transpose(
                    ps_aT[:K1, 1, :], aN[:, K0:K], id_t[:BLK, :BLK]
                )
                aTb = zp.tile([128, 2, BLK], f16, tag="atb")
                nc.scalar.copy(out=aTb, in_=ps_aT)

                za0 = zp.tile([128, BLK, BLK], f16, tag="za0")
                nc.vector.tensor_mul(
                    out=za0,
                    in0=aTb[:, 0, :].unsqueeze(2).broadcast_to([128, BLK, BLK]),
                    in1=m32h_t,
                )
                za1 = zp.tile([K1, BLK, BLK], f16, tag="za1")
                nc.vector.tensor_mul(
                    out=za1,
                    in0=aTb[:K1, 1, :].unsqueeze(2).broadcast_to(
                        [K1, BLK, BLK]
                    ),
                    in1=m32h_t[:K1],
                )
                return b0, xs, za0, za1

            def weighted_phase(wstate):
                b0, xs, za0, za1 = wstate
                ps_o = pso.tile([BLK, 256], f32, tag="pso")
                for i in range(BLK):
                    x0, x1 = xs[i // HB]
                    j = i % HB
                    nc.tensor.matmul(
                        ps_o, za0[:, i, :], x0[:, j, :],
                        start=(i == 0), stop=False,
                    )
                    nc.tensor.matmul(
                        ps_o, za1[:, i, :], x1[:, j, :],
                        start=False, stop=(i == BLK - 1),
                    )
                out_s = outp.tile([BLK, 256], f32, tag="outs")
                nc.vector.tensor_copy(out=out_s[:, 0:128], in_=ps_o[:, 0:128])
                nc.scalar.copy(out=out_s[:, 128:256], in_=ps_o[:, 128:256])
                nc.gpsimd.dma_start(
                    out=ofull[b0 : b0 + BLK, :], in_=out_s
                )

            # ---- software-pipelined main loop ----
            xpre = {0: load_xw(0), 1: load_xw(1)}
            sm_prev = None
            z_next = build_z(0)
            for bb in range(NBLK):
                z_cur = z_next
                b0, ps_a, xs, lm_t, w_ready = alpha_phase(
                    bb, z_cur, sm_prev, xpre.pop(bb)
                )
                if bb + 1 < NBLK:
                    z_next = build_z(bb + 1)
                if w_ready is not None:
                    weighted_phase(w_ready)
                sm_prev = softmax_phase((b0, ps_a, xs, lm_t))
                if bb + 2 < NBLK:
                    xpre[bb + 2] = load_xw(bb + 2)
            weighted_phase(prep_weighted(sm_prev))

    nc.finalize()
    return nc


_NC_CACHE = None


def _get_nc():
    global _NC_CACHE
    if _NC_CACHE is None:
        _NC_CACHE = gen_kernel()
    return _NC_CACHE


def build_in_maps(target_feats, neighbor_feats, neighbor_label, hist_mask, W):
    target_feats = np.ascontiguousarray(target_feats, dtype=np.float32)
    neighbor_feats = np.ascontiguousarray(neighbor_feats, dtype=np.float32)
    neighbor_label = np.ascontiguousarray(neighbor_label, dtype=np.float32)
    W = np.ascontiguousarray(W, dtype=np.float32)

    # [128, 8, D]: contiguous per-partition runs (one descriptor each)
    wt_full = np.ascontiguousarray(
        W.T.reshape(8, 128, D).transpose(1, 0, 2)
    ).astype(np.float16)
    lmask_full = np.where(np.asarray(hist_mask) > 0, 0.0, NEG).astype(np.float32)
    ident = np.eye(128, dtype=np.float16)
    m32 = np.zeros((128, BLK, BLK), dtype=np.float16)
    for i in range(BLK):
        m32[:, i, i] = 1.0

    in_maps = []
    for c in range(NCORES):
        s = slice(c * BC, (c + 1) * BC)
        xh = np.empty((K, BC, 2 * D), dtype=np.float16)
        xh[:, :, 0:D] = neighbor_feats[s].transpose(1, 0, 2)
        xh[:, :, D : 2 * D] = neighbor_label[s].transpose(1, 0, 2)
        in_maps.append({
            "tft": np.ascontiguousarray(
                target_feats[s].T.reshape(8, 128, BC).transpose(1, 0, 2)
            ).astype(np.float16),
            "wt": wt_full,
            "xh": xh,
            "lmask": lmask_full[s],
            "ident": ident,
            "m32h": m32,
        })
    return in_maps


def kernel(target_feats, neighbor_feats, neighbor_label, hist_mask, W):
    from concourse.bass_utils import run_bass_kernel_spmd

    in_maps = build_in_maps(
        target_feats, neighbor_feats, neighbor_label, hist_mask, W
    )
    nc = _get_nc()
    res = run_bass_kernel_spmd(nc, in_maps, list(range(NCORES))).results

    ofull = np.concatenate([res[c]["ofull"] for c in range(NCORES)], axis=0)
    return np.ascontiguousarray(ofull[:, 0:D]), np.ascontiguousarray(
        ofull[:, D : 2 * D]
    )
